# revision 18
# baseline (speedup 1.0000x reference)
"""DKVMN (DeepIRT) forward kernel for 8 trn2 NeuronCores.

Strategy (pure data parallel over batch, 32 samples/core):
  Host: embedding lookups folded into PRE-ACTIVATED gate tables
        (softmax/sigmoid/tanh applied to [N_Q,*] tables, then gathered),
        packed into scan-friendly device layouts, fp16. Gate streams are
        kept compact (w:100, e:50, a:50 values/step/partition) -- bulk
        pre-expansion was tried and is HBM-bandwidth-bound on real HW.
  Device per core:
    - sequential scan over S=1024 steps; per-sample state Mv [50,200]
      lives in SBUF as one [128, 2500] fp16 tile:
        partition p = v4*32 + b_local   (v4 = v // 50)
        free      f = m*50 + (v % 50)
      Per step, 5 DVE tensor_tensor ops (all in fp16 2x mode):
        T   = W (x) Mv          (w broadcast over v, dup-pair trick)
        U   = T (x) E           (e broadcast over m)
        V   = Mv - U
        WA  = W (x) A
        Mv' = V + WA
      read_t = reduce_m(T) via a 7-op contiguous fp16 halving tree on
      DVE (fp16 2x mode halves tree cost vs the fp32 original; Pool/ACT
      offload and op interleaving were measured slower on real HW).
    - prediction MLP batched after the scan on TensorE/ACT from the
      read vectors staged in DRAM (fp16).
Output: (preds [256,1024] fp32, zeros, zeros, zeros) matching reference.
"""

import contextlib

import numpy as np

MEM, KDIM, VDIM, FC = 50, 50, 200, 50
B, S_FULL = 256, 1024
NCORES = 8
BL = B // NCORES  # 32


def _sigmoid(x):
    return 1.0 / (1.0 + np.exp(-x))


def _host_prep(inputs, S):
    """Build per-core device input maps (numpy, fp16 layouts)."""
    f32 = np.float32
    q_embed_w = np.asarray(inputs["q_embed_w"], f32)
    qa_embed_w = np.asarray(inputs["qa_embed_w"], f32)
    key_memory = np.asarray(inputs["key_memory"], f32)
    init_vm = np.asarray(inputs["init_value_memory"], f32)
    erase_w = np.asarray(inputs["erase_w"], f32)
    erase_b = np.asarray(inputs["erase_b"], f32)
    add_w = np.asarray(inputs["add_w"], f32)
    add_b = np.asarray(inputs["add_b"], f32)
    pred_w1 = np.asarray(inputs["pred_w1"], f32)
    pred_b1 = np.asarray(inputs["pred_b1"], f32)
    pred_w2 = np.asarray(inputs["pred_w2"], f32)
    pred_b2 = np.asarray(inputs["pred_b2"], f32)

    q = np.clip(np.asarray(inputs["q_data"]), 0, q_embed_w.shape[0] - 1)[:, :S]
    qa = np.clip(np.asarray(inputs["qa_data"]), 0, qa_embed_w.shape[0] - 1)[:, :S]

    # Pre-activated tables (tiny BLAS + transcendentals on tables only).
    wlog = q_embed_w @ key_memory.T                      # [NQ+1, 50]
    wlog -= wlog.max(-1, keepdims=True)
    we = np.exp(wlog)
    w_tab = (we / we.sum(-1, keepdims=True)).astype(np.float16)
    hq_tab = (q_embed_w @ pred_w1[:, VDIM:].T).astype(np.float16)   # [NQ+1, 50]
    er_tab = _sigmoid(qa_embed_w @ erase_w.T + erase_b).astype(np.float16)
    ad_tab = np.tanh(qa_embed_w @ add_w.T + add_b).astype(np.float16)

    # Mv0 in scan layout [128, 2500] (replicated across b)
    mv0 = init_vm.reshape(MEM, 4, 50).transpose(1, 0, 2).reshape(4, MEM * 50)
    mv0 = np.broadcast_to(mv0[:, None, :], (4, BL, MEM * 50)).reshape(128, MEM * 50)
    mv0 = np.ascontiguousarray(mv0, dtype=np.float16)

    w1rt = np.ascontiguousarray(
        pred_w1[:, :VDIM].T.reshape(2, 100, FC), dtype=np.float16
    )  # [2, 100, 50] : [h, vp, fc]
    w2d = np.ascontiguousarray(pred_w2[0].reshape(FC, 1), dtype=np.float16)
    b1d = np.ascontiguousarray(pred_b1.reshape(FC, 1), dtype=f32)
    b2d = np.ascontiguousarray(pred_b2.reshape(1, 1), dtype=f32)

    in_maps = []
    for c in range(NCORES):
        bs = slice(c * BL, (c + 1) * BL)
        qc, qac = q[bs], qa[bs]
        w_bl = w_tab[qc]            # [32, S, 50] fp16
        e_bl = er_tab[qac]          # [32, S, 200]
        a_bl = ad_tab[qac]
        hq_bl = hq_tab[qc]          # [32, S, 50]

        # W2d [128, S*100]: [v4*32+b, t*100 + m*2 + pair]
        w2_ = np.repeat(w_bl, 2, axis=-1)                      # [32, S, 100]
        W2d = np.broadcast_to(w2_[None], (4, BL, S, 100)).reshape(128, S * 100)
        # Ed/Ad [128, S*50]: [v4*32+b, t*50+v50]
        Ed = e_bl.reshape(BL, S, 4, 50).transpose(2, 0, 1, 3).reshape(128, S * 50)
        Ad = a_bl.reshape(BL, S, 4, 50).transpose(2, 0, 1, 3).reshape(128, S * 50)
        # HQd [50, BL*S]
        HQd = hq_bl.transpose(2, 0, 1).reshape(FC, BL * S)

        in_maps.append(
            {
                "w2gate": np.ascontiguousarray(W2d),
                "egate": np.ascontiguousarray(Ed),
                "agate": np.ascontiguousarray(Ad),
                "mv0": mv0,
                "hq": np.ascontiguousarray(HQd),
                "w1rt": w1rt,
                "w2mlp": w2d,
                "b1": b1d,
                "b2": b2d,
            }
        )
    return in_maps


def build_program(S=S_FULL, chunk=64):
    """Build the Bass program (shared by all 8 cores, SPMD)."""
    import concourse.bacc as bacc
    import concourse.mybir as mybir
    from concourse.tile import TileContext

    fp16 = mybir.dt.float16
    fp32 = mybir.dt.float32
    AF = mybir.ActivationFunctionType
    OP = mybir.AluOpType

    assert S % chunk == 0
    nchunks = S // chunk
    NCOLS = BL * S            # read/pred column space (b*S + t)
    TW = min(512, S)          # MLP column tile
    assert S % TW == 0

    nc = bacc.Bacc(None, target_bir_lowering=False)

    w2g = nc.dram_tensor("w2gate", [128, S * 100], fp16, kind="ExternalInput")
    eg = nc.dram_tensor("egate", [128, S * 50], fp16, kind="ExternalInput")
    ag = nc.dram_tensor("agate", [128, S * 50], fp16, kind="ExternalInput")
    mv0d = nc.dram_tensor("mv0", [128, 2500], fp16, kind="ExternalInput")
    hqd = nc.dram_tensor("hq", [FC, NCOLS], fp16, kind="ExternalInput")
    w1rtd = nc.dram_tensor("w1rt", [2, 100, FC], fp16, kind="ExternalInput")
    w2md = nc.dram_tensor("w2mlp", [FC, 1], fp16, kind="ExternalInput")
    b1d = nc.dram_tensor("b1", [FC, 1], fp32, kind="ExternalInput")
    b2d = nc.dram_tensor("b2", [1, 1], fp32, kind="ExternalInput")
    preds_out = nc.dram_tensor("preds", [1, NCOLS], fp32, kind="ExternalOutput")
    # read vectors staged v-major: [v, b*S + t] fp16
    read_dram = nc.dram_tensor("read_scratch", [VDIM, NCOLS], fp16)

    import concourse.bass as bass

    with TileContext(nc) as tc, contextlib.ExitStack() as ctx:
        const_pool = ctx.enter_context(tc.tile_pool(name="const", bufs=1))
        state_pool = ctx.enter_context(tc.tile_pool(name="state", bufs=1))
        gate_pool = ctx.enter_context(tc.tile_pool(name="gates", bufs=2))
        read_pool = ctx.enter_context(tc.tile_pool(name="read", bufs=2))
        mlp_pool = ctx.enter_context(tc.tile_pool(name="mlp", bufs=3))
        psum_pool = ctx.enter_context(tc.tile_pool(name="psum", bufs=4, space="PSUM"))

        # ---- persistent small tiles ----
        w1r_sb = [
            const_pool.tile([100, FC], fp16, tag="w1r0", name="w1r0"),
            const_pool.tile([100, FC], fp16, tag="w1r1", name="w1r1"),
        ]
        nc.sync.dma_start(out=w1r_sb[0][:, :], in_=w1rtd[0, :, :])
        nc.sync.dma_start(out=w1r_sb[1][:, :], in_=w1rtd[1, :, :])
        w2_sb = const_pool.tile([FC, 1], fp16, tag="w2m")
        nc.sync.dma_start(out=w2_sb[:, :], in_=w2md[:, :])
        b1_sb = const_pool.tile([FC, 1], fp32, tag="b1")
        nc.sync.dma_start(out=b1_sb[:, :], in_=b1d[:, :])
        b2_sb = const_pool.tile([1, 1], fp32, tag="b2")
        nc.sync.dma_start(out=b2_sb[:, :], in_=b2d[:, :])

        # ---- state (ping-pong) ----
        mv_t = [
            state_pool.tile([128, 2500], fp16, tag="mv_a", name="mv_a"),
            state_pool.tile([128, 2500], fp16, tag="mv_b", name="mv_b"),
        ]
        nc.sync.dma_start(out=mv_t[0][:, :], in_=mv0d[:, :])

        def view4(ap2d):  # [128,2500] -> [128, m, v25, pair]
            return ap2d.rearrange("p (m v25 two) -> p m v25 two", m=MEM, v25=25, two=2)

        # persistent scan scratch (serial chain reuses them every step)
        tt = state_pool.tile([128, 2500], fp16, tag="tt", name="tt")
        uu = state_pool.tile([128, 2500], fp16, tag="uu", name="uu")
        vv = state_pool.tile([128, 2500], fp16, tag="vv", name="vv")
        wa = state_pool.tile([128, 2500], fp16, tag="wa", name="wa")
        # fp16 read-tree scratch (all on DVE; fp16 2x mode halves tree cost)
        th = state_pool.tile([128, 1250], fp16, tag="th", name="th")
        t2 = state_pool.tile([128, 600], fp16, tag="t2", name="t2")
        t3 = state_pool.tile([128, 300], fp16, tag="t3", name="t3")
        t4 = state_pool.tile([128, 150], fp16, tag="t4", name="t4")
        t5 = state_pool.tile([128, 50], fp16, tag="t5", name="t5")
        t6 = state_pool.tile([128, 50], fp16, tag="t6", name="t6")

        def wv_view(gtile, k):
            return (
                gtile[:, k * 100:(k + 1) * 100]
                .rearrange("p (m two) -> p m two", m=MEM, two=2)
                .unsqueeze(2)
                .broadcast_to((128, MEM, 25, 2))
            )

        def gv_view(gtile, k):  # e/a gates: [p, v25, two] broadcast over m
            return (
                gtile[:, k * 50:(k + 1) * 50]
                .rearrange("p (v25 two) -> p v25 two", v25=25, two=2)
                .unsqueeze(1)
                .broadcast_to((128, MEM, 25, 2))
            )

        # ================= scan =================
        for c in range(nchunks):
            w2c = gate_pool.tile([128, chunk * 100], fp16, tag="w2c", name="w2c")
            ec = gate_pool.tile([128, chunk * 50], fp16, tag="ec", name="ec")
            ac = gate_pool.tile([128, chunk * 50], fp16, tag="ac", name="ac")
            nc.sync.dma_start(out=w2c[:, :], in_=w2g[:, c * chunk * 100:(c + 1) * chunk * 100])
            nc.sync.dma_start(out=ec[:, :], in_=eg[:, c * chunk * 50:(c + 1) * chunk * 50])
            nc.sync.dma_start(out=ac[:, :], in_=ag[:, c * chunk * 50:(c + 1) * chunk * 50])
            rdc = read_pool.tile([128, 50 * chunk], fp16, tag="rdc", name="rdc")
            rdc3 = rdc[:, :].rearrange("p (v50 tc) -> p v50 tc", v50=50, tc=chunk)

            for k in range(chunk):
                t = c * chunk + k
                cur, nxt = mv_t[t % 2], mv_t[(t + 1) % 2]

                nc.vector.tensor_tensor(out=view4(tt[:, :]), in0=view4(cur[:, :]), in1=wv_view(w2c, k), op=OP.mult)
                nc.vector.tensor_tensor(out=view4(uu[:, :]), in0=view4(tt[:, :]), in1=gv_view(ec, k), op=OP.mult)
                nc.vector.tensor_sub(vv[:, :], cur[:, :], uu[:, :])
                nc.vector.tensor_tensor(out=view4(wa[:, :]), in0=wv_view(w2c, k), in1=gv_view(ac, k), op=OP.mult)
                nc.vector.tensor_add(nxt[:, :], vv[:, :], wa[:, :])
                # read_t = sum_m T via contiguous fp16 binary tree (2x mode)
                nc.vector.tensor_add(th[:, :], tt[:, :1250], tt[:, 1250:2500])   # 25 m'
                nc.vector.tensor_add(t2[:, :], th[:, :600], th[:, 600:1200])     # 12
                nc.vector.tensor_add(t3[:, :], t2[:, :300], t2[:, 300:600])      # 6
                nc.vector.tensor_add(t4[:, :], t3[:, :150], t3[:, 150:300])      # 3
                nc.vector.tensor_add(t5[:, :], t4[:, :50], t4[:, 50:100])        # +pair
                nc.vector.tensor_add(t6[:, :], t5[:, :], t4[:, 100:150])         # +odd3
                nc.vector.tensor_add(rdc3[:, :, k], t6[:, :], th[:, 1200:1250])  # +carry25

            # write chunk reads to DRAM v-major (4 HWDGE dma, one per v4)
            for v4 in range(4):
                src = rdc[v4 * BL:(v4 + 1) * BL, :].rearrange(
                    "p (v50 tc) -> p v50 tc", v50=50, tc=chunk
                )
                dst = bass.AP(
                    read_dram,
                    (v4 * 50) * NCOLS + c * chunk,
                    [[S, BL], [NCOLS, 50], [1, chunk]],
                )
                nc.sync.dma_start(out=dst, in_=src)

        # ================= prediction MLP =================
        for b in range(BL):
            for ti in range(S // TW):
                col0 = b * S + ti * TW
                rd0 = mlp_pool.tile([100, TW], fp16, tag="rd0", name="rd0")
                rd1 = mlp_pool.tile([100, TW], fp16, tag="rd1", name="rd1")
                nc.sync.dma_start(
                    out=rd0[:, :],
                    in_=bass.AP(read_dram, col0, [[NCOLS, 100], [1, TW]]),
                )
                nc.sync.dma_start(
                    out=rd1[:, :],
                    in_=bass.AP(read_dram, 100 * NCOLS + col0, [[NCOLS, 100], [1, TW]]),
                )
                hqt = mlp_pool.tile([FC, TW], fp16, tag="hqt", name="hqt")
                nc.sync.dma_start(out=hqt[:, :], in_=hqd[:, col0:col0 + TW])

                ph = psum_pool.tile([FC, TW], fp32, tag="ph", name="ph")
                nc.tensor.matmul(ph[:, :], lhsT=w1r_sb[0][:, :], rhs=rd0[:, :], start=True, stop=False)
                nc.tensor.matmul(ph[:, :], lhsT=w1r_sb[1][:, :], rhs=rd1[:, :], start=False, stop=True)

                hsum = mlp_pool.tile([FC, TW], fp32, tag="hsum", name="hsum")
                nc.vector.tensor_add(hsum[:, :], ph[:, :], hqt[:, :])
                htan = mlp_pool.tile([FC, TW], fp16, tag="htan", name="htan")
                nc.scalar.activation(htan[:, :], hsum[:, :], AF.Tanh, bias=b1_sb[:, :])

                pl = psum_pool.tile([1, TW], fp32, tag="pl", name="pl")
                nc.tensor.matmul(pl[:, :], lhsT=w2_sb[:, :], rhs=htan[:, :], start=True, stop=True)
                psb = mlp_pool.tile([1, TW], fp32, tag="psb", name="psb")
                nc.scalar.activation(psb[:, :], pl[:, :], AF.Sigmoid, bias=b2_sb[:, :])
                nc.sync.dma_start(out=preds_out[0:1, col0:col0 + TW], in_=psb[:, :])

    nc.compile()
    return nc


def kernel(**inputs):
    S = np.asarray(inputs["q_data"]).shape[1]
    in_maps = _host_prep(inputs, S)
    nc = build_program(S=S, chunk=min(64, S))

    from concourse.bass_utils import run_bass_kernel_spmd

    res = run_bass_kernel_spmd(nc, in_maps, core_ids=list(range(NCORES)))
    preds = np.zeros((B, S), np.float32)
    for c in range(NCORES):
        preds[c * BL:(c + 1) * BL] = res.results[c]["preds"].reshape(BL, S)
    z = np.zeros_like(preds)
    return (preds, z, z, z)


if __name__ == "__main__":
    import pickle

    with open("/tmp/inputs.pkl", "rb") as f:
        I = pickle.load(f)
    out = kernel(**I)
    exp = np.load("/tmp/expected0.npy")
    err = np.abs(out[0] - exp)
    print("abs err max", err.max(), "mean", err.mean())
